# revision 8
# baseline (speedup 1.0000x reference)
"""Trainium2 Bass kernel for: out = segment_sum(sigmoid(x @ w), segment_ids).

Shapes (hardcoded): x [1048576, 64] f32, w [64, 128] f32,
segment_ids [1048576] int32 (sorted), num_segments = 4096. Output [4096, 128] f32.

Strategy (8 cores, data parallel by bags):
  - 4096 bags -> 512 bags/core -> 16 windows of 32 bags per core.
  - Window items padded to NBW blocks of 128 items; blocks grouped G=12
    (3 f32 PSUM banks), double-buffered; pad blocks are skipped everywhere.
  - mm1 (per block): lhsT = xT block [64, 128] bf16 row-paired (block p on
    partitions 0-63, p+6 on 64-127 -> concurrent row-group matmuls),
    rhs = replicated w bf16 -> z [128 items, 128 C] f32 PSUM.
  - sigmoid z -> s bf16: on ScalarE (one ACTIVATE per group, trimmed to
    real blocks), except group 1 in every window, which runs a
    degree-5 odd polynomial on the otherwise-idle VectorE.  Its
    mm2 is deferred to the window end so the slow DVE chain never stalls
    the tensor engine.
  - mm2 (per block): lhsT = host-built onehot [128 items, 32 bags] bf16,
    rhs = s block -> accumulate out [32 bags, 128 C] f32 PSUM per window.
  - A PE warmup burst runs under the first DMA so HAM reaches 2.4 GHz
    before the real matmuls start.
"""

import os
from collections import deque

import numpy as np
import ml_dtypes

# problem constants (hardcoded per harness contract)
N = 1048576
F = 64
C = 128
B = 4096
NC = 8           # cores
BPC = B // NC    # bags per core = 512
W = 32           # bags per window
NW = BPC // W    # windows per core = 16
BLK = 128        # items per block
G = 12           # blocks per PSUM group (3 f32 banks)
H = G // 2       # pair offset within group
DVE_BLKS = G     # all of group 1's sigmoid runs on VectorE

bf16 = ml_dtypes.bfloat16

# sigmoid(z) ~= 0.5 + z*(C1 + t*(C3 + t*C5)), t = z^2, z clamped to
# [-8, 8]; density-weighted fit (zero bias under N(0,1) inputs).
C1 = 2.35907128e-01
C3 = -8.01316207e-03
C5 = 9.02781750e-05


def _host_prepare(x, w, segment_ids):
    counts = np.bincount(segment_ids, minlength=B)
    off = np.zeros(B + 1, np.int64)
    off[1:] = np.cumsum(counts)
    starts = off[:-1:W][: NC * NW]
    ends = off[W::W][: NC * NW]
    per_win = (ends - starts).astype(np.int64)
    maxblk = int(-(-per_win.max() // BLK))
    NBW = G * (-(-maxblk // G))
    NP2 = NBW // 2

    x_bf = x.astype(bf16)
    w_bf = w.astype(bf16)
    iota = np.arange(W, dtype=np.int32)

    in_maps = []
    nreal = np.zeros((NC, NW), np.int64)
    for k in range(NC):
        XS = np.zeros((NW, 128, NP2 * BLK), bf16)
        OH = np.zeros((NW, 128, NBW * W), bf16)
        for wi in range(NW):
            widx = k * NW + wi
            i0, i1 = int(starts[widx]), int(ends[widx])
            n = i1 - i0
            nreal[k, wi] = -(-n // BLK)
            xb = np.zeros((NBW * BLK, F), bf16)
            xb[:n] = x_bf[i0:i1]
            xb3 = xb.reshape(NBW, BLK, F).transpose(0, 2, 1)  # [NBW, 64, 128]
            cols = np.empty((128, NP2, BLK), bf16)
            idx_top = (np.arange(NP2) // H) * G + (np.arange(NP2) % H)
            cols[0:64] = xb3[idx_top].transpose(1, 0, 2)
            cols[64:128] = xb3[idx_top + H].transpose(1, 0, 2)
            XS[wi] = cols.reshape(128, NP2 * BLK)

            sa = np.full((NBW * BLK,), -1, np.int32)
            sa[:n] = segment_ids[i0:i1] - (widx * W)
            oh3 = (sa.reshape(NBW, BLK)[:, :, None] == iota).astype(bf16)
            OH[wi] = oh3.transpose(1, 0, 2).reshape(128, NBW * W)
        in_maps.append({
            "xs": XS,
            "oh": OH,
            "wrep": np.concatenate([w_bf, w_bf], axis=0),
        })
    return in_maps, NBW, nreal


def _build_bass(NBW, nr_prog):
    import concourse.bass as bass
    import concourse.bacc as bacc
    import concourse.tile as tile
    from concourse import mybir

    NP2 = NBW // 2
    alu = mybir.AluOpType
    nc = bacc.Bacc("TRN2", target_bir_lowering=False, debug=False)
    XS = nc.dram_tensor("xs", [NW, 128, NP2 * BLK], mybir.dt.bfloat16,
                        kind="ExternalInput")
    OH = nc.dram_tensor("oh", [NW, 128, NBW * W], mybir.dt.bfloat16,
                        kind="ExternalInput")
    WREP = nc.dram_tensor("wrep", [128, C], mybir.dt.bfloat16,
                          kind="ExternalInput")
    OUT = nc.dram_tensor("out", [NW, W, C], mybir.dt.float32,
                         kind="ExternalOutput")

    DV = DVE_BLKS * BLK

    with tile.TileContext(nc) as tc:
        from contextlib import ExitStack
        with ExitStack() as ctx:
            const_pool = ctx.enter_context(tc.tile_pool(name="const", bufs=1))
            xs_pool = ctx.enter_context(tc.tile_pool(name="xs", bufs=3))
            oh_pool = ctx.enter_context(tc.tile_pool(name="oh", bufs=3))
            s_pool = ctx.enter_context(tc.tile_pool(name="s", bufs=3))
            sdve_pool = ctx.enter_context(tc.tile_pool(name="sdve", bufs=2))
            sig_pool = ctx.enter_context(tc.tile_pool(name="sig", bufs=8))
            out_sb_pool = ctx.enter_context(tc.tile_pool(name="osb", bufs=2))
            z_ps_pool = ctx.enter_context(
                tc.tile_pool(name="zps", bufs=2, space="PSUM"))
            out_ps_pool = ctx.enter_context(
                tc.tile_pool(name="ops", bufs=2, space="PSUM"))

            wrep_sb = const_pool.tile([128, C], mybir.dt.bfloat16)

            pending = deque()

            for wi in range(NW):
                nr = int(nr_prog[wi])
                ng = -(-nr // G)
                xs = xs_pool.tile([128, NP2 * BLK], mybir.dt.bfloat16,
                                  tag="xs")
                nc.gpsimd.dma_start(xs[:, :ng * H * BLK],
                                    XS[wi, :, :ng * H * BLK])
                oh = oh_pool.tile([128, NBW * W], mybir.dt.bfloat16, tag="oh")
                nc.sync.dma_start(oh[:, :nr * W], OH[wi, :, :nr * W])
                if wi == 0:
                    # wrep DMA after xs(w0) so the first window isn't delayed;
                    # PE warmup burst (~3.8us cold) flips HAM to 2.4 GHz while
                    # the first x window streams in.
                    nc.gpsimd.dma_start(wrep_sb[:], WREP[:])
                    warm = z_ps_pool.tile([128, G * BLK], mybir.dt.float32,
                                          tag="z")
                    for _ in range(12):
                        nc.tensor.matmul(warm[:, 0:C], lhsT=wrep_sb[:],
                                         rhs=wrep_sb[:], start=True,
                                         stop=True, skip_group_check=True)
                out_ps = out_ps_pool.tile([W, C], mybir.dt.float32, tag="op")
                s_dve = sdve_pool.tile([128, DV], mybir.dt.bfloat16,
                                       tag="sd")

                for g in range(ng):
                    real_g = min(G, nr - G * g)
                    z = z_ps_pool.tile([128, G * BLK], mybir.dt.float32,
                                       tag="z")
                    for p in range(H):
                        j0 = G * g + p
                        j1 = j0 + H
                        c0 = (H * g + p) * BLK
                        if j0 < nr:
                            nc.tensor.matmul(
                                z[:, p * BLK:(p + 1) * BLK],
                                lhsT=xs[0:64, c0:c0 + BLK],
                                rhs=wrep_sb[0:64, :],
                                start=True, stop=True)
                        if j1 < nr:
                            nc.tensor.matmul(
                                z[:, (p + H) * BLK:(p + H + 1) * BLK],
                                lhsT=xs[64:128, c0:c0 + BLK],
                                rhs=wrep_sb[64:128, :],
                                start=True, stop=True)

                    s = s_pool.tile([128, G * BLK], mybir.dt.bfloat16,
                                    tag="s")
                    nfd = real_g * BLK
                    if g == 1:
                        # whole group -> VectorE deg-5 polynomial sigmoid
                        u = sig_pool.tile([128, DV], mybir.dt.bfloat16,
                                          tag="u")
                        t = sig_pool.tile([128, DV], mybir.dt.bfloat16,
                                          tag="t")
                        ha = sig_pool.tile([128, DV], mybir.dt.bfloat16,
                                           tag="ha")
                        hb = sig_pool.tile([128, DV], mybir.dt.bfloat16,
                                           tag="hb")
                        nc.vector.tensor_scalar(u[:], z[:, :DV], -8.0, 8.0,
                                                alu.max, alu.min)
                        nc.vector.tensor_tensor(out=t[:], in0=u[:], in1=u[:],
                                                op=alu.mult)
                        nc.vector.tensor_scalar(ha[:], t[:], C5, C3,
                                                alu.mult, alu.add)
                        nc.vector.tensor_tensor(out=hb[:], in0=ha[:],
                                                in1=t[:], op=alu.mult)
                        nc.vector.tensor_scalar_add(ha[:], hb[:], C1)
                        nc.vector.tensor_tensor(out=hb[:], in0=ha[:],
                                                in1=u[:], op=alu.mult)
                        nc.vector.tensor_scalar_add(s_dve[:], hb[:], 0.5)
                    else:
                        nc.scalar.activation(
                            s[:, :nfd], z[:, :nfd],
                            mybir.ActivationFunctionType.Sigmoid)

                    def mm2_group(s=s, oh=oh, out_ps=out_ps, g=g,
                                  real_g=real_g, nr=nr):
                        if g == 1:
                            return
                        for q in range(real_g):
                            j = G * g + q
                            nc.tensor.matmul(
                                out_ps[:],
                                lhsT=oh[:, j * W:(j + 1) * W],
                                rhs=s[:, q * BLK:(q + 1) * BLK],
                                start=(j == 0),
                                stop=False,
                                skip_group_check=True)
                    pending.append(mm2_group)

                    while len(pending) > 2:
                        pending.popleft()()

                def finish_window(s_dve=s_dve, oh=oh, out_ps=out_ps, wi=wi):
                    # deferred mm2 for the VectorE-sigmoid group (blocks G..2G-1)
                    for q in range(DVE_BLKS):
                        j = G + q
                        nc.tensor.matmul(
                            out_ps[:],
                            lhsT=oh[:, j * W:(j + 1) * W],
                            rhs=s_dve[:, q * BLK:(q + 1) * BLK],
                            start=False,
                            stop=(q == DVE_BLKS - 1),
                            skip_group_check=True)
                    out_sb = out_sb_pool.tile([W, C], mybir.dt.float32,
                                              tag="osb")
                    nc.vector.tensor_copy(out_sb[:], out_ps[:])
                    nc.sync.dma_start(OUT[wi], out_sb[:])
                pending.append(finish_window)

            while pending:
                pending.popleft()()

    nc.finalize()
    return nc


def kernel(x, w, segment_ids, num_segments):
    x = np.ascontiguousarray(np.asarray(x, dtype=np.float32))
    w = np.ascontiguousarray(np.asarray(w, dtype=np.float32))
    segment_ids = np.ascontiguousarray(np.asarray(segment_ids, dtype=np.int32))
    assert int(num_segments) == B
    assert x.shape == (N, F) and w.shape == (F, C)

    from concourse.bass_utils import run_bass_kernel_spmd

    in_maps, NBW, nreal = _host_prepare(x, w, segment_ids)
    nr_prog = nreal.max(axis=0)          # SPMD: shared program, max per window
    assert nr_prog.min() >= 2 * G, "window too small for DVE offload layout"
    nc = _build_bass(NBW, nr_prog)

    trace = os.environ.get("KERNEL_TRACE", "0") == "1"
    res = run_bass_kernel_spmd(nc, in_maps, core_ids=list(range(NC)),
                               trace=trace)
    if trace and res.exec_time_ns is not None:
        print(f"HW exec time: {res.exec_time_ns} ns")

    out = np.concatenate(
        [r["out"].reshape(BPC, C) for r in res.results], axis=0)
    return np.ascontiguousarray(out.astype(np.float32))
